# revision 32
# baseline (speedup 1.0000x reference)
"""Trainium2 Bass kernel for nn_Experts (grouped MoE expert MLP).

Computes, for each of 8 experts e:
    h   = x_e @ w0_e.T          # [2048,1024] @ [1024,4096] -> [2048,4096]
    g   = gelu_exact(h)
    out = g @ w3_e.T            # [2048,4096] @ [4096,1024] -> [2048,1024]
then masks unpopular experts with zero gating activity (output_tensor).

Sharding: expert-parallel, 1 expert per NeuronCore across 8 cores (SPMD —
one compiled NEFF, per-core input data).

Numerics/perf strategy: fp8(e4m3) DoubleRow matmuls. DoubleRow processes two
128-deep k-tiles per instruction at 0.5 cycles/output-column — 4x the bf16
MAC rate. Plain fp8 quantization (~4% per element) would blow the 2e-2
rel-err budget, so every GEMM runs a 3-pass residual-corrected product

    a @ b ~= a_hi @ b_hi + a_hi @ b_lo + a_lo @ b_hi

where v_hi = e4m3(v), v_lo = e4m3(v - v_hi), all accumulated in one fp32
PSUM group. That is 3x the MACs of one fp8 pass but still 0.75x the bf16
cycle count, with end-to-end rel err ~2e-3 (vs 3.3e-3 for bf16).

The full 3-pass scheme underspends the error budget 10x, so slivers of the
correction passes are skipped along the CONTRACTION axis (where the omitted
quantization noise averages down by sqrt(skipped fraction) in every output
element, instead of concentrating in a subset of outputs): GEMM1's x_lo
pass drops its last dc-pair (1/4 of that correction) and GEMM2's g_lo pass
drops its last two fc-pairs (1/8). Measured end-to-end rel err 1.77e-2,
under the 2e-2 gate, for 11 GEMM1 matmuls per h tile (vs 12 full / 16
bf16-equivalent) and 46 GEMM2 matmuls per out tile (vs 48 / 64).

Scaling: x is pre-scaled by 16 and w0/w3 by 256 (powers of two) so both the
hi values and the residuals sit in e4m3's normal range; the descales fold
into the GELU activation input scale (1/4096) and the output copy (1/256).

Layouts are contraction-major and grouped by DMA unit (as in the bf16 v2
kernel) so every load has long contiguous per-partition runs:
    xT  [128, 4 tb, 8 dc, 512 t]   (hi+lo)
    w0T [128, 8 g, 8 dc, 512 f]    (hi+lo)
    w3T [128, 32 fc, 1024 d]       (hi+lo)
GEMM1 emits h tiles [f=128, t=512] in PSUM; GELU (scaled) moves them to SBUF
as f32; Pool quantizes g_hi, DVE computes the g_lo residual, both into
fc-paired e4m3 tiles [128, 2, 512] that are directly GEMM2's stationary
operand slices.
"""

import numpy as np
import ml_dtypes

T = 2048      # tokens (capacity) per expert
D = 1024      # hidden
F = 4096      # ffn
P = 128       # partitions
DC = D // P   # 8 d k-tiles (GEMM1 contraction)
FC = F // P   # 32 f k-tiles (GEMM2 contraction)
DW = 512      # GEMM2 output free-dim chunk
TBS = 512     # token block
NTBS = T // TBS   # 4
NTS = TBS // P    # 4
FCG = 4       # fc per w0/w3 DMA group
G = FC // FCG     # 8 f-groups
FW = FCG * P      # 512
NUM_LOCAL = 4
N_CORES = 8

XS = 16.0     # x pre-scale (power of two)
WS = 256.0    # w0/w3 pre-scale (power of two)

_cache = {}


def _build_nc_fp8(
    g_extra=1,           # spare fc-pair tile slots beyond FC//2
    h_bufs=3,
    o_ps_bufs=2,
    o_sb_bufs=3,
    x_bufs=2,            # per hi/lo x stream
    gf_bufs=3,           # f32 gelu staging tiles
    warmup_mms=10,       # scratch matmuls so the PE rides out the cold-clock
                         # window during the initial DMA wait
):
    import sys
    if "/opt/trn_rl_repo" not in sys.path:
        sys.path.insert(0, "/opt/trn_rl_repo")
    import concourse.tile as tile
    import concourse.mybir as mybir
    from concourse import bacc

    f8 = mybir.dt.float8e4
    bf16 = mybir.dt.bfloat16
    f32 = mybir.dt.float32
    AFT = mybir.ActivationFunctionType
    DR = mybir.MatmulPerfMode.DoubleRow

    nc = bacc.Bacc(
        "TRN2",
        target_bir_lowering=False,
        debug=False,
        enable_asserts=True,
        num_devices=N_CORES,
    )

    xh = nc.dram_tensor("xh", [P, NTBS, DC, TBS], f8, kind="ExternalInput").ap()
    xl = nc.dram_tensor("xl", [P, NTBS, DC, TBS], f8, kind="ExternalInput").ap()
    w0h = nc.dram_tensor("w0h", [P, G, DC, FW], f8, kind="ExternalInput").ap()
    w0l = nc.dram_tensor("w0l", [P, G, DC, FW], f8, kind="ExternalInput").ap()
    w3h = nc.dram_tensor("w3h", [P, FC, D], f8, kind="ExternalInput").ap()
    w3l = nc.dram_tensor("w3l", [P, FC, D], f8, kind="ExternalInput").ap()
    out = nc.dram_tensor("out", [T, D], f32, kind="ExternalOutput").ap()

    with tile.TileContext(nc) as tc:
        with (
            tc.tile_pool(name="weights", bufs=1) as wpool,
            tc.tile_pool(name="xin", bufs=2 * x_bufs) as xpool,
            tc.tile_pool(name="ghi", bufs=FC // 2 + g_extra) as ghpool,
            tc.tile_pool(name="glo", bufs=FC // 2 + g_extra) as glpool,
            tc.tile_pool(name="gf32", bufs=gf_bufs) as gfpool,
            tc.tile_pool(name="ostage", bufs=o_sb_bufs) as opool,
            tc.tile_pool(name="hps", bufs=h_bufs, space="PSUM") as hpsum,
            tc.tile_pool(name="ops", bufs=o_ps_bufs, space="PSUM") as opsum,
            tc.tile_pool(name="tailops", bufs=2, space="PSUM") as tailpsum,
        ):
            w0h_sb = wpool.tile([P, G, DC, FW], f8, name="w0h_sb", tag="w0h_sb")
            w0l_sb = wpool.tile([P, G, DC, FW], f8, name="w0l_sb", tag="w0l_sb")
            w3h_sb = wpool.tile([P, FC, D], f8, name="w3h_sb", tag="w3h_sb")
            w3l_sb = wpool.tile([P, FC, D], f8, name="w3l_sb", tag="w3l_sb")

            x_tiles = {}

            XLDC = DC - 2   # x_lo's last dc-pair is never consumed (skipped pass)

            def load_x(tb):
                th = xpool.tile([P, DC, TBS], f8, name=f"xh_{tb}", tag="xh")
                tl = xpool.tile([P, XLDC, TBS], f8, name=f"xl_{tb}", tag="xl")
                nc.sync.dma_start(th[:], xh[:, tb])
                nc.sync.dma_start(tl[:], xl[:, tb, :XLDC])
                x_tiles[tb] = (th, tl)

            if warmup_mms:
                with (
                    tc.tile_pool(name="warm", bufs=1) as warmpool,
                    tc.tile_pool(name="warmps", bufs=1, space="PSUM") as warmpsum,
                ):
                    wsrc = warmpool.tile([P, DW], bf16, name="wsrc", tag="wsrc")
                    wps = warmpsum.tile([P, DW], f32, name="wps", tag="wps")
                    nc.vector.memset(wsrc[:], 0.0)
                    # taper the last few so the warmup ends right when the
                    # first real operands land (finer alignment than 427ns)
                    widths = [DW] * (warmup_mms - 2) + [DW // 2] * 2 + [DW // 4]
                    for w in widths:
                        nc.tensor.matmul(wps[:, :w], wsrc[:, :P], wsrc[:, :w],
                                         start=True, stop=True)

            # critical prefix, ordered by first consumption: fc0's passes run
            # (w0h,xh), (w0h,xl), (w0l,xh), so interleave tb0's x streams with
            # w0 group 0; remaining w0 groups follow in GEMM1 order; w3
            # streams in behind (phase B of tb0 starts ~48us in).
            th0 = xpool.tile([P, DC, TBS], f8, name="xh_0", tag="xh")
            tl0 = xpool.tile([P, XLDC, TBS], f8, name="xl_0", tag="xl")
            nc.sync.dma_start(th0[:], xh[:, 0])
            nc.sync.dma_start(w0h_sb[:, 0], w0h[:, 0])
            nc.sync.dma_start(tl0[:], xl[:, 0, :XLDC])
            nc.sync.dma_start(w0l_sb[:, 0], w0l[:, 0])
            x_tiles[0] = (th0, tl0)
            for g in range(1, G):
                nc.sync.dma_start(w0h_sb[:, g], w0h[:, g])
                nc.sync.dma_start(w0l_sb[:, g], w0l[:, g])
            # halves interleaved h/l: phase B of tb0 reads the w3h pass first
            # and the w3l pass ~3us later; this ordering lands each just in
            # time instead of serializing all of w3h before any w3l
            nc.sync.dma_start(w3h_sb[:, :FC // 2], w3h[:, :FC // 2])
            nc.sync.dma_start(w3h_sb[:, FC // 2:], w3h[:, FC // 2:])
            nc.sync.dma_start(w3l_sb[:, :FC // 2], w3l[:, :FC // 2])
            nc.sync.dma_start(w3l_sb[:, FC // 2:], w3l[:, FC // 2:])

            for tb in range(NTBS):
                if tb + 1 < NTBS:
                    load_x(tb + 1)
                xh_t, xl_t = x_tiles.pop(tb)

                # phase A: GEMM1 (3-pass DoubleRow) + GELU + hi/lo quantize
                gh_tiles, gl_tiles = [], []
                for fcp in range(FC // 2):
                    gh_tiles.append(ghpool.tile([P, 2, TBS], f8,
                                                name=f"gh_{tb}_{fcp}", tag="gh"))
                    gl_tiles.append(glpool.tile([P, 2, TBS], f8,
                                                name=f"gl_{tb}_{fcp}", tag="gl"))
                for fc in range(FC):
                    g_, j = fc // FCG, fc % FCG
                    h_ps = hpsum.tile([P, TBS], f32, name=f"h_{tb}_{fc}", tag="h_ps")
                    # x_lo pass skips its last dc-pair: the residual error of
                    # the skipped quarter averages down by sqrt(1/4) across
                    # the contraction, trading ~1.1e-2 of the 2e-2 rel-err
                    # budget for 1/12 of GEMM1's matmuls
                    g1_passes = ((w0h_sb, xh_t, DC // 2),
                                 (w0h_sb, xl_t, DC // 2 - 1),
                                 (w0l_sb, xh_t, DC // 2))
                    total_mm = sum(n for _, _, n in g1_passes)
                    n_mm = 0
                    for w_sb, x_t, ndp in g1_passes:
                        for dp in range(ndp):
                            nc.tensor.matmul(
                                h_ps[:],
                                w_sb[:, g_, 2 * dp:2 * dp + 2,
                                     j * P:(j + 1) * P],
                                x_t[:, 2 * dp:2 * dp + 2],
                                start=(n_mm == 0),
                                stop=(n_mm == total_mm - 1),
                                perf_mode=DR,
                            )
                            n_mm += 1
                    g_f = gfpool.tile([P, TBS], f32, name=f"gf_{tb}_{fc}", tag="gf")
                    nc.scalar.activation(g_f[:], h_ps[:], AFT.Gelu,
                                         scale=1.0 / (XS * WS))
                    nc.gpsimd.tensor_copy(gh_tiles[fc // 2][:, fc % 2], g_f[:])
                    if fc // 2 < FC // 2 - 2:  # gl of last two fc-pairs unused
                        nc.vector.tensor_sub(gl_tiles[fc // 2][:, fc % 2],
                                             g_f[:],
                                             gh_tiles[fc // 2][:, fc % 2])

                # phase B: GEMM2 (3-pass DoubleRow), one [t=128, d=512] psum
                # accumulation group at a time. Pass order puts the w3l pass
                # last (its DMA is the final prefix transfer, so tb0's first
                # group stalls least) and the fcp15 matmuls of every pass at
                # the group's very end (fc31's gelu->quantize chain finishes
                # ~2us after phase A's last matmul).
                FCP = FC // 2
                passes = ((gh_tiles, w3h_sb), (gl_tiles, w3h_sb),
                          (gh_tiles, w3l_sb))
                # the g_lo pass also skips its last two fc-pairs (1/8 of that
                # correction, same sqrt-averaging argument as in GEMM1).
                # Emission order: w3h passes first, w3l's second half (f8+)
                # dead last — tb0's first group otherwise stalls on the w3l_b
                # prefix transfer; the fc15 (gh-tail) matmuls sit late too,
                # covering the gelu->quantize latency of fc30/31.
                order = ([(0, f) for f in range(FCP - 1)]
                         + [(1, f) for f in range(FCP - 2)]
                         + [(2, f) for f in range(FCP // 2)]
                         + [(0, FCP - 1)]
                         + [(2, f) for f in range(FCP // 2, FCP - 1)]
                         + [(2, FCP - 1)])

                def emit_ogroup(o_ps, ts, dlo, dwid):
                    for i, (p, fcp) in enumerate(order):
                        g_t, w_sb = passes[p]
                        nc.tensor.matmul(
                            o_ps[:],
                            g_t[fcp][:, :, ts * P:(ts + 1) * P],
                            w_sb[:, 2 * fcp:2 * fcp + 2, dlo:dlo + dwid],
                            start=(i == 0),
                            stop=(i == len(order) - 1),
                            perf_mode=DR,
                        )

                for ts in range(NTS):
                    for dc2 in range(2):
                        rows = slice(tb * TBS + ts * P, tb * TBS + (ts + 1) * P)
                        if tb == NTBS - 1 and ts == NTS - 1 and dc2 == 1:
                            # final group: two half-width chunks so the first
                            # chunk's descale+DMA overlaps the second's
                            # matmuls, shortening the kernel tail
                            o_sb = opool.tile([P, DW], f32, name="os_tail",
                                              tag="o_sb")
                            HW2 = DW // 4
                            for c in range(4):
                                dlo = dc2 * DW + c * HW2
                                o_ps = tailpsum.tile([P, HW2], f32,
                                                     name=f"o_t_{c}", tag="o_t")
                                emit_ogroup(o_ps, ts, dlo, HW2)
                                nc.vector.tensor_scalar_mul(
                                    o_sb[:, c * HW2:(c + 1) * HW2], o_ps[:],
                                    1.0 / WS)
                                nc.sync.dma_start(
                                    out[rows, dlo:dlo + HW2],
                                    o_sb[:, c * HW2:(c + 1) * HW2])
                        else:
                            o_ps = opsum.tile([P, DW], f32,
                                              name=f"o_{tb}_{ts}_{dc2}",
                                              tag="o_ps")
                            emit_ogroup(o_ps, ts, dc2 * DW, DW)
                            o_sb = opool.tile([P, DW], f32,
                                              name=f"os_{tb}_{ts}_{dc2}",
                                              tag="o_sb")
                            nc.vector.tensor_scalar_mul(o_sb[:], o_ps[:],
                                                        1.0 / WS)
                            nc.sync.dma_start(out[rows, dc2 * DW:(dc2 + 1) * DW],
                                              o_sb[:])

    nc.compile()
    return nc


def _get_nc():
    if "nc" not in _cache:
        _cache["nc"] = _build_nc_fp8()
    return _cache["nc"]


def _make_cached_fn(nc):
    """Build a reusable jitted 8-core executable around bass2jax's bass_exec
    primitive (the same lowering run_bass_kernel_spmd uses under axon), so
    repeat kernel() calls skip retrace/relower."""
    import jax
    import numpy as np
    from jax.sharding import Mesh, PartitionSpec
    try:
        from jax.experimental.shard_map import shard_map
    except ImportError:
        from jax.shard_map import shard_map
    import concourse.mybir as mybir
    from concourse.bass2jax import (_bass_exec_p, install_neuronx_cc_hook,
                                    partition_id_tensor)

    install_neuronx_cc_hook()
    partition_name = nc.partition_id_tensor.name if nc.partition_id_tensor else None
    in_names, out_names, out_avals, zero_shapes = [], [], [], []
    for alloc in nc.m.functions[0].allocations:
        if not isinstance(alloc, mybir.MemoryLocationSet):
            continue
        name = alloc.memorylocations[0].name
        if alloc.kind == "ExternalInput":
            if name != partition_name:
                in_names.append(name)
        elif alloc.kind == "ExternalOutput":
            out_names.append(name)
            shape = tuple(alloc.tensor_shape)
            dtype = mybir.dt.np(alloc.dtype)
            out_avals.append(jax.core.ShapedArray(shape, dtype))
            zero_shapes.append((shape, dtype))
    n_params = len(in_names)
    all_in_names = list(in_names) + list(out_names)
    if partition_name is not None:
        all_in_names.append(partition_name)

    def _body(*args):
        ins = list(args[:n_params])
        outs = list(args[n_params:])
        extra = [partition_id_tensor()] if partition_name is not None else []
        return tuple(_bass_exec_p.bind(
            *ins, *outs, *extra,
            out_avals=tuple(out_avals),
            in_names=tuple(all_in_names),
            out_names=tuple(out_names),
            lowering_input_output_aliases=(),
            sim_require_finite=True,
            sim_require_nnan=True,
            nc=nc,
        ))

    devices = jax.devices()[:N_CORES]
    mesh = Mesh(np.asarray(devices), ("core",))
    fn = jax.jit(
        shard_map(_body, mesh=mesh,
                  in_specs=(PartitionSpec("core"),) * (n_params + len(out_names)),
                  out_specs=(PartitionSpec("core"),) * len(out_names),
                  check_rep=False),
        keep_unused=True)

    def run(in_maps):
        concat_in = [np.concatenate([np.asarray(m[n]) for m in in_maps], axis=0)
                     for n in in_names]
        concat_zeros = [np.zeros((N_CORES * s[0], *s[1:]), dt)
                        for s, dt in zero_shapes]
        outs = fn(*concat_in, *concat_zeros)
        return [
            {name: np.asarray(outs[i]).reshape(N_CORES, *out_avals[i].shape)[c]
             for i, name in enumerate(out_names)}
            for c in range(N_CORES)
        ]

    return run


def kernel(**inputs):
    import os
    import sys
    if "/opt/trn_rl_repo" not in sys.path:
        sys.path.insert(0, "/opt/trn_rl_repo")
    from concourse import bass_utils

    output_tensor = np.asarray(inputs["output_tensor"], dtype=np.float32)  # [1, 8]
    x = np.asarray(inputs["inputs"], dtype=np.float32)   # [1, 8, 2048, 1024]
    w0 = np.asarray(inputs["w0"], dtype=np.float32)      # [8, 4096, 1024]
    w3 = np.asarray(inputs["w3"], dtype=np.float32)      # [8, 1024, 4096]

    E4 = ml_dtypes.float8_e4m3

    def hi_lo(a):
        hi = a.astype(E4)
        lo = (a - hi.astype(np.float32)).astype(E4)
        return hi, lo

    def prep_expert(e):
        # scale (powers of two; descale folded into gelu/output scales), then
        # split into e4m3 hi + e4m3 residual, then transpose the 1-byte
        # tensors into contraction-major grouped layouts:
        #   xT  [128, 4 tb, 8 dc, 512 t],  w0T [128, 8 g, 8 dc, 512 f],
        #   w3T [128, 32 fc, 1024 d]
        xe_h, xe_l = hi_lo(x[0, e] * XS)          # [t, d]
        w0_h, w0_l = hi_lo(w0[e] * WS)            # [f, d]
        w3_h, w3_l = hi_lo(w3[e] * WS)            # [d, f]

        def xT(a):
            return np.ascontiguousarray(
                a.T.reshape(DC, P, NTBS, TBS).transpose(1, 2, 0, 3))

        def w0T(a):
            return np.ascontiguousarray(
                a.T.reshape(DC, P, G, FW).transpose(1, 2, 0, 3))

        def w3T(a):
            return np.ascontiguousarray(
                a.T.reshape(FC, P, D).transpose(1, 0, 2))

        return {
            "xh": xT(xe_h), "xl": xT(xe_l),
            "w0h": w0T(w0_h), "w0l": w0T(w0_l),
            "w3h": w3T(w3_h), "w3l": w3T(w3_l),
        }

    from concurrent.futures import ThreadPoolExecutor
    with ThreadPoolExecutor(max_workers=N_CORES) as pool:
        in_maps = list(pool.map(prep_expert, range(N_CORES)))

    nc = _get_nc()
    results = None
    if "fast_fn" in _cache:
        try:
            results = _cache["fast_fn"](in_maps)
        except Exception:
            results = None
    if results is None:
        try:
            results = bass_utils.run_bass_kernel_spmd(
                nc, in_maps, core_ids=list(range(N_CORES))).results
        except ModuleNotFoundError:
            # trace path requested via env but axon NTFF hook missing
            os.environ["BASS_NEVER_TRACE"] = "1"
            results = bass_utils.run_bass_kernel_spmd(
                nc, in_maps, core_ids=list(range(N_CORES))).results
        try:
            fast = _make_cached_fn(nc)
            fast(in_maps)  # warm: jit trace + XLA/NEFF compile happens here
            _cache["fast_fn"] = fast
        except Exception:
            pass
    out_full = np.stack([results[e]["out"] for e in range(N_CORES)])[None]

    # unpopular experts with zero gating activity produce zeros
    unpop = output_tensor[:, NUM_LOCAL:].sum(axis=0) != 0
    mask = np.concatenate([np.ones(NUM_LOCAL, dtype=bool), unpop])
    out_full = out_full * mask[None, :, None, None].astype(np.float32)
    return out_full.astype(np.float32)
